# revision 1
# baseline (speedup 1.0000x reference)
"""Trainium2 Bass kernel for nn_CritiGraph (ct_val expansion).

Math: ct_val[b,t1,t2,m,tp] = (dis_sum - dis_sta_pos + dis_cnc_pos)/TP with
dis(c1,c2,norm) = sign(c1)sign(c2) * (1 - table[|c1|^|c2|]) * norm and
table[x] = (floor(log2(x+1))+1)/16.  Since 1-table[X] = (15-e)/16 with
e = floor(log2(X+1)) = exponent field of float(X+1), the table gather
reduces to: XOR -> +1 (int->f32 value cast) -> shift 23 -> subtract, all
elementwise.  Candidate magnitudes factor as |cnc| = ori ^ fm with
fm[h*K+k] = 2^h | (rand & (2^h-1)) (and fm=0 for the ori slot), so
|cnc|^|pos| = (ori^|pos|) ^ fm = base[t2,tp] ^ fm[m,tp].

Device layout per core (8 of 64 tokens, data-parallel):
  2 supertiles x [partition = 4 tokens * 32 t2 = 128, free = (m,tp)].
  fm rows are DMA-broadcast 32x across each token's partitions; base /
  scale / bias are per-partition [128,8] scalars applied per-tp slice.
  ScalarE finishes with out = S*scale + bias (S = e-15).

Candidate sign is structural in m (perm of [+1*1025, -1*1024]); the rare
result==0 exceptions (negated candidate that is actually 0, sign +1) are
patched on the host afterward.
"""

import os
from contextlib import ExitStack

import numpy as np

import concourse.bacc as bacc
import concourse.mybir as mybir
import concourse.tile as tile

H = 16
TP = 8
K = 64
M = 2 * H * K + 1  # 2049
B, T1, T2 = 4, 16, 32
NTOK = B * T1      # 64
NCORE = 8
TPC = NTOK // NCORE   # tokens per core = 8
TOK_ST = 4            # tokens per supertile (4*32 = 128 partitions)
NST = TPC // TOK_ST   # supertiles per core = 2
FW = M * TP           # free width = 16392

F32 = mybir.dt.float32
I32 = mybir.dt.int32


def _exp_log2p1(x):
    """floor(log2(x+1)) for integer array x >= 0, exact via f64 frexp."""
    return (np.frexp((np.asarray(x, np.int64) + 1).astype(np.float64))[1] - 1).astype(
        np.int32
    )


def _host_prep(sta_loc, pos_loc, val_n, rand_raw, perm):
    f32 = np.float32
    sta = np.asarray(sta_loc).reshape(NTOK, TP)
    pos = np.asarray(pos_loc)                      # [B,T2,TP]
    valn = np.asarray(val_n, np.float32).reshape(NTOK, T2)
    perm = np.asarray(perm).astype(np.int64)

    ori = np.abs(sta).astype(np.int64)             # [NTOK,TP]
    ssign = np.where(sta >= 0, f32(1.0), f32(-1.0))
    posmag = np.abs(pos).astype(np.int64)          # [B,T2,TP]
    psign = np.where(pos >= 0, f32(1.0), f32(-1.0))

    # fm candidate xor-deltas: [NTOK, H, K, TP] -> [NTOK, 1024, TP]
    hbits = np.arange(H, dtype=np.int64)
    fm_pre = ((np.int64(1) << hbits)[None, :, None, None]
              | (np.asarray(rand_raw) & ((np.int64(1) << hbits) - 1)[None, :, None, None]
                 )).reshape(NTOK, H * K, TP)
    fm_cat = np.concatenate(
        [fm_pre, np.zeros((NTOK, 1, TP), np.int64), fm_pre], axis=1)   # [NTOK,M,TP]
    sgn_cat = np.concatenate(
        [np.ones(H * K + 1, np.float32), -np.ones(H * K, np.float32)])
    fm_all = fm_cat[:, perm, :].astype(np.int32)   # [NTOK,M,TP]
    sgn_m = sgn_cat[perm]                          # [M] structural candidate sign

    # exceptions: structurally-negated candidate whose value is 0 (sign +1)
    exc = np.argwhere((sgn_m[None, :, None] < 0)
                      & (fm_all == ori[:, None, :].astype(np.int32)))

    # host distances sta<->pos (tiny), mirroring reference f32 order
    pm_tok = posmag[np.arange(NTOK) // T1]         # [NTOK,T2,TP]
    ps_tok = psign[np.arange(NTOK) // T1]          # [NTOK,T2,TP]
    e_sp = _exp_log2p1(ori[:, None, :] ^ pm_tok)
    s_sp = ((e_sp + 1).astype(np.float32) / f32(H))
    dis_sta = (ssign[:, None, :] * ps_tok) * (f32(1.0) - s_sp) * valn[:, :, None]
    dis_sum = dis_sta.sum(axis=-1, dtype=np.float32)
    A = dis_sum[:, :, None] - dis_sta              # [NTOK,T2,TP] f32
    bias = A * f32(1.0 / TP)
    scale_p = -(ps_tok * valn[:, :, None]) * f32(1.0 / (H * TP))  # for sgn=+1 runs
    base = (ori[:, None, :] ^ pm_tok).astype(np.int32)            # [NTOK,T2,TP]

    return dict(fm_all=fm_all, sgn_m=sgn_m, exc=exc, base=base,
                bias=bias, scale_p=scale_p, pm_tok=pm_tok, ps_tok=ps_tok,
                valn=valn, A=A)


def _runs_of_sign(sgn_m):
    """[(start, end, sign), ...] contiguous runs of sgn_m."""
    runs = []
    s = 0
    for i in range(1, M + 1):
        if i == M or sgn_m[i] != sgn_m[s]:
            runs.append((s, i, float(sgn_m[s])))
            s = i
    return runs


def _chunks_for(sgn_m, max_w=544):
    """(c0, c1, sign) chunks.  If few sign runs, split at run boundaries so
    each chunk is single-sign (sign applied via ScalarE scale -> no DVE
    multiply).  Otherwise sign=None chunks (device multiplies by sgn row)."""
    runs = _runs_of_sign(sgn_m)
    if len(runs) <= 8:
        chunks = []
        for s, e, g in runs:
            for c0 in range(s, e, max_w):
                chunks.append((c0, min(c0 + max_w, e), g))
        return chunks, True
    chunks = [(c0, min(c0 + max_w, M), None) for c0 in range(0, M, max_w)]
    return chunks, False


def _build_program(chunks, path_a, reps=1):
    nc = bacc.Bacc("TRN2", target_bir_lowering=False, debug=False)

    fm_h = nc.dram_tensor("fm", [TPC, FW], I32, kind="ExternalInput")
    base_h = nc.dram_tensor("base", [NST, 128, TP], I32, kind="ExternalInput")
    scalep_h = nc.dram_tensor("scalep", [NST, 128, TP], F32, kind="ExternalInput")
    scalen_h = (nc.dram_tensor("scalen", [NST, 128, TP], F32, kind="ExternalInput")
                if path_a else None)
    bias_h = nc.dram_tensor("bias", [NST, 128, TP], F32, kind="ExternalInput")
    biasn_h = (nc.dram_tensor("biasn", [NST, 128, TP], F32, kind="ExternalInput")
               if path_a else None)
    sgn_h = None if path_a else nc.dram_tensor("sgn", [M], F32, kind="ExternalInput")
    out_h = nc.dram_tensor("out", [NST, 128, FW], F32, kind="ExternalOutput")

    with tile.TileContext(nc) as tc, ExitStack() as ctx:
        cpool = ctx.enter_context(tc.tile_pool(name="consts", bufs=1))
        fmpool = ctx.enter_context(tc.tile_pool(name="fm", bufs=3))
        opool = ctx.enter_context(tc.tile_pool(name="outs", bufs=3))

        base_t = cpool.tile([128, NST * TP], I32)
        scalep_t = cpool.tile([128, NST * TP], F32)
        bias_t = cpool.tile([128, NST * TP], F32)
        for st in range(NST):
            nc.sync.dma_start(base_t[:, st * TP:(st + 1) * TP], base_h.ap()[st])
            nc.sync.dma_start(scalep_t[:, st * TP:(st + 1) * TP], scalep_h.ap()[st])
            nc.sync.dma_start(bias_t[:, st * TP:(st + 1) * TP], bias_h.ap()[st])
        if path_a:
            scalen_t = cpool.tile([128, NST * TP], F32)
            biasn_t = cpool.tile([128, NST * TP], F32)
            for st in range(NST):
                nc.sync.dma_start(scalen_t[:, st * TP:(st + 1) * TP],
                                  scalen_h.ap()[st])
                nc.sync.dma_start(biasn_t[:, st * TP:(st + 1) * TP],
                                  biasn_h.ap()[st])
        else:
            sgn_t = cpool.tile([128, M], F32)
            nc.sync.dma_start(
                sgn_t[:], sgn_h.ap().unsqueeze(0).to_broadcast((128, M)))

        def one_chunk(st, c0, c1, g):
            mw = c1 - c0
            L = mw * TP
            fm_t = fmpool.tile([128, L], I32, tag="fm")
            src = (fm_h.ap()[st * TOK_ST:(st + 1) * TOK_ST, c0 * TP:c1 * TP]
                   .unsqueeze(1).to_broadcast((TOK_ST, T2, L)))
            nc.sync.dma_start(fm_t[:], src)

            fm3 = fm_t[:].rearrange("p (m t) -> p m t", t=TP)
            base_b = (base_t[:, st * TP:(st + 1) * TP]
                      .unsqueeze(1).to_broadcast((128, mw, TP)))
            nc.vector.tensor_tensor(
                fm3, fm3, base_b, mybir.AluOpType.bitwise_xor)

            fview = fm_t[:].bitcast(F32)
            # float(X+1): int ALU input, fp add, f32-converted writeback
            nc.vector.tensor_scalar(
                fview, fm_t[:], 1, None, mybir.AluOpType.add)
            # bits >> 23 = e + 127 (bitwise: i32 -> i32, no cast allowed);
            # the -142 is folded into the host ACT bias (path A) or the
            # STT below (path B).
            nc.vector.tensor_scalar(
                fm_t[:], fm_t[:], 23, None, mybir.AluOpType.logical_shift_right)
            s3 = fview.rearrange("p (m t) -> p m t", t=TP)
            if path_a:
                # arith converter: i32 (e+127) -> f32, in place
                nc.vector.tensor_copy(fview, fm_t[:])
            else:
                sgn_b = (sgn_t[:, c0:c1].unsqueeze(2)
                         .to_broadcast((128, mw, TP)))
                i3 = fm_t[:].rearrange("p (m t) -> p m t", t=TP)
                nc.vector.scalar_tensor_tensor(
                    s3, i3, 142.0, sgn_b,
                    mybir.AluOpType.subtract, mybir.AluOpType.mult)

            out_t = opool.tile([128, L], F32, tag="out")
            o3 = out_t[:].rearrange("p (m t) -> p m t", t=TP)
            sc_t = scalep_t if (g is None or g > 0) else scalen_t
            bi_t = bias_t if (g is None or g > 0) else biasn_t
            for tp in range(TP):
                j = st * TP + tp
                nc.scalar.activation(
                    o3[:, :, tp], s3[:, :, tp],
                    mybir.ActivationFunctionType.Identity,
                    bias=bi_t[:, j:j + 1], scale=sc_t[:, j:j + 1])
            nc.sync.dma_start(out_h.ap()[st, :, c0 * TP:c1 * TP], out_t[:])

        for _rep in range(reps):
            for st in range(NST):
                for (c0, c1, g) in chunks:
                    one_chunk(st, c0, c1, g)

    nc.compile()
    return nc


def _in_maps(prep, path_a):
    """Per-core input dicts."""
    fm_all, base = prep["fm_all"], prep["base"]
    bias, scale_p = prep["bias"], prep["scale_p"]
    maps = []
    for c in range(NCORE):
        t0 = c * TPC
        d = {
            "fm": fm_all[t0:t0 + TPC].reshape(TPC, FW).astype(np.int32),
            "base": base[t0:t0 + TPC].reshape(NST, 128, TP).astype(np.int32),
            "scalep": scale_p[t0:t0 + TPC].reshape(NST, 128, TP),
            "bias": bias[t0:t0 + TPC].reshape(NST, 128, TP),
        }
        if path_a:
            d["scalen"] = -d["scalep"]
            # ACT input is e+127 on path A; fold the -142 into the bias
            d["biasn"] = d["bias"] - np.float32(142.0) * d["scalen"]
            d["bias"] = d["bias"] - np.float32(142.0) * d["scalep"]
        else:
            d["sgn"] = prep["sgn_m"].astype(np.float32)
        maps.append(d)
    return maps


def _apply_exceptions(out, prep):
    """Overwrite columns where a structurally-negated candidate is 0."""
    f32 = np.float32
    for tok, m, tp in prep["exc"]:
        pm = prep["pm_tok"][tok, :, tp]            # [T2]
        ps = prep["ps_tok"][tok, :, tp]
        e0 = _exp_log2p1(pm)
        s0 = (e0 + 1).astype(np.float32) / f32(H)
        dis_cnc = ps * (f32(1.0) - s0) * prep["valn"][tok]
        out[tok, :, m, tp] = (prep["A"][tok, :, tp] + dis_cnc) * f32(1.0 / TP)
    return out


def kernel(sta_loc, pos_loc, val_n, rand_raw, perm, _sim=False):
    prep = _host_prep(sta_loc, pos_loc, val_n, rand_raw, perm)
    chunks, path_a = _chunks_for(prep["sgn_m"])
    nc = _build_program(chunks, path_a)
    maps = _in_maps(prep, path_a)

    if _sim:
        from concourse.bass_interp import CoreSim
        results = []
        for c in range(NCORE):
            sim = CoreSim(nc, trace=False)
            for k, v in maps[c].items():
                sim.tensor(k)[:] = v
            sim.simulate(check_with_hw=False)
            results.append({"out": np.array(sim.tensor("out"))})
    else:
        from concourse.bass_utils import run_bass_kernel_spmd
        res = run_bass_kernel_spmd(nc, maps, list(range(NCORE)))
        results = res.results

    out = np.empty((NTOK, T2, M, TP), np.float32)
    for c in range(NCORE):
        o = results[c]["out"].reshape(NST, TOK_ST, T2, FW)
        for st in range(NST):
            tok0 = c * TPC + st * TOK_ST
            out[tok0:tok0 + TOK_ST] = o[st].reshape(TOK_ST, T2, M, TP)
    out = _apply_exceptions(out, prep)
    return out.reshape(B, T1, T2, M, TP)


if __name__ == "__main__":
    pass

